# revision 3
# baseline (speedup 1.0000x reference)
"""TRN2 Bass kernel for nn_BaseAttention_46548855554192.

B=2, S=2048, H=2048, NH=16, HD=128 multi-head attention with RoPE and an
additive attention mask, computed tensor-parallel over heads on 8 NeuronCores
(2 heads per core).  Each core computes qkv projection for its heads, RoPE,
causal softmax attention, and a partial o_proj (its head-columns of o_w);
the host sums the 8 partial outputs.

Layout strategy (per core):
  - hidden is fed transposed hT [H, B*S]; qkv computed as qkvT [feat, s] so
    the head dim (128) lands on SBUF partitions.
  - scores are computed transposed: scoresT [k, q] = kT_blk.T-free matmul,
    which makes the softmax sum a partition-dim reduction done on the PE
    (ones-matmul) and exp a PSUM->SBUF ACT op.
  - no max-subtraction in softmax: scores are O(10) for randn inputs, and
    exp(f32) is safe up to ~88.  (Reference subtracts max, mathematically
    identical.)
  - v is computed in [s, d] orientation directly so PV needs no transposes.
  - attnT [d, q] is normalized during PSUM evacuation by a broadcast
    reciprocal (computed as exp(-ln(sum)) on ACT).
  - o_proj emits out [s, e] so the output DMA is contiguous and row-norm
    could ride per-partition; RoPE uses host-precomputed cos/sin tables
    (ACT Sin has no range reduction) and a signed-permutation matmul for
    rotate_half (no cross-partition engine ops exist).
  - all matmuls use float32r (~1.5e-4 rel err, 4x faster than float32).
"""

import numpy as np

import bass_rust
import concourse.bass as bass
import concourse.mybir as mybir
from concourse.tile import TileContext
from concourse.vector_clock import ScopedClock

F32 = mybir.dt.float32
F32R = mybir.dt.float32r
AF = mybir.ActivationFunctionType
OP = mybir.AluOpType

B, S, H, NH, HD = 2, 2048, 2048, 16, 128
BS = B * S                  # 4096
HPC = NH // 8               # heads per core = 2
DLOC = HPC * HD             # local attn dims = 256
CH = 256                    # chunk / q-block width
NCH = S // CH               # 8 chunks per batch
KT = S // 128               # 16 k-tiles of 128 per batch
SCALE = 1.0 / float(np.sqrt(HD))
ROPE_BASE = 10000.0

MAX_WAITS = 1  # this container's walrus supports one sync-wait per instruction


class PatchedTileContext(TileContext):
    """Split multi-sem waits into single-wait NOPs (old-walrus limitation)."""

    def _lower_ordered_insts(self, ordered):
        for bb_name, insts in ordered.items():
            new_list = []
            for inst in insts:
                si = inst.sync_info
                if si is not None and len(si.on_wait) > MAX_WAITS:
                    waits = list(si.on_wait)
                    keep = waits[:MAX_WAITS]
                    extra = waits[MAX_WAITS:]
                    scopes = self._inst_to_scopes.get(inst.name, ())
                    for i in range(0, len(extra), MAX_WAITS):
                        group = extra[i:i + MAX_WAITS]
                        nop = mybir.InstNoOp(
                            name=f"waitsplit-{self.nc.next_id()}",
                            engine=inst.engine,
                            sync_info=mybir.SyncInfo(on_wait=list(group), on_update=[]),
                            bass_nofuse=True,
                        )
                        self._inst_to_scopes[nop.name] = scopes
                        new_list.append(nop)
                    inst.sync_info = bass_rust.SyncInfo(
                        on_wait=keep, on_update=list(si.on_update)
                    )
                new_list.append(inst)
            insts[:] = new_list
        return super()._lower_ordered_insts(ordered)

    def _drain_and_barrier(self, tick_clock, wait_clock):
        nc = self.nc
        drain_inst = nc.sync.drain()
        wait_clock.add_sem_waits(
            drain_inst.ins, ScopedClock({None: tick_clock.global_clock})
        )
        si = drain_inst.ins.sync_info
        waits = list(si.on_wait) if si is not None else []
        if len(waits) > MAX_WAITS:
            assert self.sems is not None
            by_name = {h.name: h for h in self.sems.allocated().values()}
            keep = waits[:MAX_WAITS]
            extra = []
            for w in waits[MAX_WAITS:]:
                h = by_name.get(w.ant_name)
                if h is None:
                    keep.append(w)
                else:
                    extra.append((h, w.wait_value, w.wait_mode))
            drain_inst.ins.sync_info = bass_rust.SyncInfo(
                on_wait=keep, on_update=list(si.on_update) if si else []
            )
            for h, val, mode in extra:
                assert mode == "sem-ge-imm", mode
                nc.sync.wait_ge(h, val)

        nc.all_engine_barrier()
        assert self.sems is not None
        popped = nc._tile_sem_poison_stack.pop()
        assert popped is self._sem_poison
        nc.clear_and_free_semaphores(list(self.sems.allocated().values()))
        nc.all_engine_barrier()


def build_kernel(mask_mode: str) -> bass.Bass:
    """mask_mode: 'causal' (skip masked tiles), 'dense' (no mask),
    'generic' (additive mask streamed from DRAM)."""
    nc = bass.Bass()

    hT = nc.dram_tensor("hT", [H, BS], F32, kind="ExternalInput")
    wqkT = nc.dram_tensor("wqkT", [H, 4 * 128], F32, kind="ExternalInput")
    wvT = nc.dram_tensor("wvT", [H, DLOC], F32, kind="ExternalInput")
    owT = nc.dram_tensor("owT", [DLOC, H], F32, kind="ExternalInput")
    bqkT = nc.dram_tensor("bqkT", [128, 4], F32, kind="ExternalInput")
    cosT = nc.dram_tensor("cosT", [128, BS], F32, kind="ExternalInput")
    sinS = nc.dram_tensor("sinS", [128, BS], F32, kind="ExternalInput")
    permP = nc.dram_tensor("permP", [128, 128], F32, kind="ExternalInput")
    ones128 = nc.dram_tensor("ones128", [128, 128], F32, kind="ExternalInput")
    if mask_mode == "causal":
        cmask0 = nc.dram_tensor("cmask0", [128, CH], F32, kind="ExternalInput")
        cmask1 = nc.dram_tensor("cmask1", [128, CH], F32, kind="ExternalInput")
    if mask_mode == "generic":
        maskT = nc.dram_tensor("maskT", [B, S, S], F32, kind="ExternalInput")
    outP = nc.dram_tensor("outP", [BS, H], F32, kind="ExternalOutput")

    with PatchedTileContext(nc) as tc:
        with (
            tc.tile_pool(name="const", bufs=1) as cpool,
            tc.tile_pool(name="work", bufs=1) as wpool,
            tc.tile_pool(name="sb", bufs=2) as sb,
            tc.tile_pool(name="mp", bufs=8) as mp,
            tc.tile_pool(name="ps", bufs=8, space="PSUM") as ps,
        ):
            # ---- resident constants ----
            wqk_t = cpool.tile([128, KT * 512], F32R, tag="wqk")
            nc.sync.dma_start(
                wqk_t[:].rearrange("p (kt m) -> p kt m", kt=KT),
                wqkT[:, :].rearrange("(kt p) m -> p kt m", p=128).bitcast(F32R),
            )
            wv_t = cpool.tile([128, KT * DLOC], F32R, tag="wv")
            nc.sync.dma_start(
                wv_t[:].rearrange("p (kt m) -> p kt m", kt=KT),
                wvT[:, :].rearrange("(kt p) m -> p kt m", p=128).bitcast(F32R),
            )
            ow_t = cpool.tile([128, 2 * H], F32R, tag="ow")
            nc.sync.dma_start(
                ow_t[:].rearrange("p (dt e) -> p dt e", dt=2),
                owT[:, :].rearrange("(dt p) e -> p dt e", p=128).bitcast(F32R),
            )
            bqk_t = cpool.tile([128, 4], F32, tag="bqk")
            nc.sync.dma_start(bqk_t[:], bqkT[:, :])
            perm_t = cpool.tile([128, 128], F32R, tag="perm")
            nc.sync.dma_start(perm_t[:], permP[:, :].bitcast(F32R))
            ones_t = cpool.tile([128, 128], F32R, tag="ones")
            nc.sync.dma_start(ones_t[:], ones128[:, :].bitcast(F32R))
            if mask_mode == "causal":
                cm0_t = cpool.tile([128, CH], F32R, tag="cm0")
                nc.sync.dma_start(cm0_t[:], cmask0[:, :].bitcast(F32R))
                cm1_t = cpool.tile([128, CH], F32R, tag="cm1")
                nc.sync.dma_start(cm1_t[:], cmask1[:, :].bitcast(F32R))

            for b in range(B):
                s_base = b * S
                # per-batch rope tables
                cos_t = wpool.tile([128, S], F32, tag="cos")
                nc.sync.dma_start(cos_t[:], cosT[:, s_base:s_base + S])
                sin_t = wpool.tile([128, S], F32, tag="sin")
                nc.sync.dma_start(sin_t[:], sinS[:, s_base:s_base + S])

                # per-batch products
                qk_t = wpool.tile([128, 4 * S], F32R, tag="qkT")   # 4 m-tiles x [128,S]
                v_t = wpool.tile([128, KT * DLOC], F32R, tag="v")  # KT s-tiles x [128,256]

                # ---------- phase 1: qkv projection ----------
                for n in range(NCH):
                    s0 = s_base + n * CH
                    h_t = sb.tile([128, KT * CH], F32R, tag="big")
                    nc.sync.dma_start(
                        h_t[:].rearrange("p (kt s) -> p kt s", kt=KT),
                        hT[:, s0:s0 + CH].rearrange("(kt p) s -> p kt s", p=128).bitcast(F32R),
                    )
                    # q,k m-tiles: out [feat,128][s,CH]
                    for m in range(4):
                        p_qk = ps.tile([128, 512], F32, tag="ps")
                        for kt in range(KT):
                            nc.tensor.matmul(
                                p_qk[:, 0:CH],
                                wqk_t[:, kt * 512 + m * 128: kt * 512 + (m + 1) * 128],
                                h_t[:, kt * CH:(kt + 1) * CH],
                                start=(kt == 0), stop=(kt == KT - 1),
                            )
                        # evacuate with bias add; q/k pre-rope values
                        nc.scalar.activation(
                            qk_t[:, m * S + n * CH: m * S + (n + 1) * CH],
                            p_qk[:, 0:CH],
                            AF.Identity,
                            bias=bqk_t[:, m:m + 1],
                        )
                    # v: out [s,128][d,256] per 128-subtile
                    for st in range(2):
                        p_v = ps.tile([128, 512], F32, tag="ps")
                        for kt in range(KT):
                            nc.tensor.matmul(
                                p_v[:, 0:DLOC],
                                h_t[:, kt * CH + st * 128: kt * CH + (st + 1) * 128],
                                wv_t[:, kt * DLOC:(kt + 1) * DLOC],
                                start=(kt == 0), stop=(kt == KT - 1),
                            )
                        stile = n * 2 + st
                        nc.scalar.activation(
                            v_t[:, stile * DLOC:(stile + 1) * DLOC],
                            p_v[:, 0:DLOC], AF.Copy,
                        )

                # ---------- rope on q,k (all 4 m-tiles) ----------
                for m in range(4):
                    for j in range(NCH):
                        c0 = j * CH
                        qk_sl = qk_t[:, m * S + c0: m * S + c0 + CH]
                        p_rot = ps.tile([128, 512], F32, tag="ps")
                        nc.tensor.matmul(
                            p_rot[:, 0:CH], perm_t[:], qk_sl,
                            start=True, stop=True,
                        )
                        rot_sb = sb.tile([128, CH], F32, tag="rot")
                        nc.scalar.activation(rot_sb[:], p_rot[:, 0:CH], AF.Copy)
                        # rot *= sinS ; t2 = qk*cos ; qk = rot + t2  (f32r out)
                        nc.vector.tensor_tensor(
                            rot_sb[:], rot_sb[:], sin_t[:, c0:c0 + CH], OP.mult
                        )
                        t2 = sb.tile([128, CH], F32, tag="t2")
                        nc.vector.tensor_tensor(
                            t2[:], qk_sl.bitcast(F32), cos_t[:, c0:c0 + CH], OP.mult
                        )
                        nc.vector.tensor_tensor(qk_sl, t2[:], rot_sb[:], OP.add)

                # ---------- phase 2: attention ----------
                for qb in range(NCH):
                    n_kt = 2 * (qb + 1) if mask_mode == "causal" else KT
                    if mask_mode == "generic":
                        mask_tiles = []
                        for pair in range(n_kt // 2):
                            mt = mp.tile([128, 512], F32, tag="mask")
                            nc.sync.dma_start(
                                mt[:].rearrange("p (t q) -> p t q", t=2),
                                maskT[b, pair * 256:(pair + 1) * 256,
                                      qb * CH:(qb + 1) * CH]
                                .rearrange("(t p) q -> p t q", p=128),
                            )
                            mask_tiles.append(mt)
                    at_tiles = []
                    for hh in range(HPC):
                        qof = hh * S
                        kof = (2 + hh) * S
                        ex_t = sb.tile([128, KT * CH], F32R, tag="big")
                        for pair in range(n_kt // 2):
                            p_sc = ps.tile([128, 512], F32, tag="ps")
                            for half in range(2):
                                kt = 2 * pair + half
                                nc.tensor.matmul(
                                    p_sc[:, half * CH:(half + 1) * CH],
                                    qk_t[:, kof + kt * 128: kof + (kt + 1) * 128],
                                    qk_t[:, qof + qb * CH: qof + (qb + 1) * CH],
                                    start=True, stop=True, skip_group_check=True,
                                )
                            if mask_mode == "generic":
                                mt = mask_tiles[pair]
                                nc.vector.tensor_tensor(
                                    p_sc[:, 0:CH], p_sc[:, 0:CH], mt[:, 0:CH], OP.add
                                )
                                nc.vector.tensor_tensor(
                                    p_sc[:, CH:2 * CH], p_sc[:, CH:2 * CH],
                                    mt[:, CH:2 * CH], OP.add
                                )
                            nc.scalar.activation(
                                ex_t[:, pair * 512:(pair + 1) * 512],
                                p_sc[:, 0:512], AF.Exp, scale=SCALE,
                            )
                        if mask_mode == "causal":
                            # staircase masking of the two diagonal k-tiles
                            nc.vector.tensor_tensor(
                                ex_t[:, 2 * qb * CH:(2 * qb + 1) * CH],
                                ex_t[:, 2 * qb * CH:(2 * qb + 1) * CH],
                                cm0_t[:], OP.mult,
                            )
                            nc.vector.tensor_tensor(
                                ex_t[:, (2 * qb + 1) * CH:(2 * qb + 2) * CH],
                                ex_t[:, (2 * qb + 1) * CH:(2 * qb + 2) * CH],
                                cm1_t[:], OP.mult,
                            )
                        # softmax denominator: ones-matmul partition reduction
                        p_sum = ps.tile([128, 512], F32, tag="ps")
                        for kt in range(n_kt):
                            nc.tensor.matmul(
                                p_sum[:, 0:CH], ones_t[:],
                                ex_t[:, kt * CH:(kt + 1) * CH],
                                start=(kt == 0), stop=(kt == n_kt - 1),
                            )
                        # PV
                        p_at = ps.tile([128, 512], F32, tag="ps")
                        for kt in range(n_kt):
                            nc.tensor.matmul(
                                p_at[:, 0:CH],
                                v_t[:, kt * DLOC + hh * 128: kt * DLOC + (hh + 1) * 128],
                                ex_t[:, kt * CH:(kt + 1) * CH],
                                start=(kt == 0), stop=(kt == n_kt - 1),
                            )
                        # 1/sum broadcast: exp(-ln(sum)); rows all equal already
                        ln_t = sb.tile([128, CH], F32, tag="ln")
                        nc.scalar.activation(ln_t[:], p_sum[:, 0:CH], AF.Ln)
                        rec_t = sb.tile([128, CH], F32, tag="rec")
                        nc.scalar.activation(rec_t[:], ln_t[:], AF.Exp, scale=-1.0)
                        at_t = sb.tile([128, CH], F32R, tag="attn")
                        nc.vector.tensor_tensor(at_t[:], p_at[:, 0:CH], rec_t[:], OP.mult)
                        at_tiles.append(at_t)

                    # ---------- o_proj for this q-block ----------
                    for ss in range(CH // 128):
                        for ec in range(H // 512):
                            p_o = ps.tile([128, 512], F32, tag="ps")
                            for hh in range(HPC):
                                nc.tensor.matmul(
                                    p_o[:],
                                    at_tiles[hh][:, ss * 128:(ss + 1) * 128],
                                    ow_t[:, hh * H + ec * 512: hh * H + (ec + 1) * 512],
                                    start=(hh == 0), stop=(hh == HPC - 1),
                                )
                            o_sb = sb.tile([128, 512], F32, tag="osb")
                            nc.scalar.activation(o_sb[:], p_o[:], AF.Copy)
                            nc.sync.dma_start(
                                outP[s_base + qb * CH + ss * 128:
                                     s_base + qb * CH + (ss + 1) * 128,
                                     ec * 512:(ec + 1) * 512],
                                o_sb[:],
                            )
    return nc


def _causal_patterns():
    p = np.arange(128)[:, None]
    j = np.arange(CH)[None, :]
    cm0 = (p <= j).astype(np.float32)          # k-tile aligned with q-block start
    cm1 = (p + 128 <= j).astype(np.float32)    # next k-tile
    return cm0, cm1


def _host_prep(hidden_states, position_ids, attention_mask, qkv_w, qkv_b, o_w):
    hidden_states = np.asarray(hidden_states, dtype=np.float32)
    position_ids = np.asarray(position_ids)
    attention_mask = np.asarray(attention_mask, dtype=np.float32)
    qkv_w = np.asarray(qkv_w, dtype=np.float32)
    qkv_b = np.asarray(qkv_b, dtype=np.float32)
    o_w = np.asarray(o_w, dtype=np.float32)

    # mask mode detection
    causal = np.triu(np.full((S, S), -1e9, dtype=np.float32), k=1)
    m = attention_mask.reshape(B, S, S)
    if all(np.array_equal(m[b], causal) for b in range(B)):
        mask_mode = "causal"
    elif not attention_mask.any():
        mask_mode = "dense"
    else:
        mask_mode = "generic"

    # rope tables
    half = HD // 2
    inv = (1.0 / ROPE_BASE ** (np.arange(half, dtype=np.float64) / half))
    freqs = position_ids.astype(np.float64).reshape(BS, 1) * inv[None, :]  # [BS,64]
    c = np.cos(freqs).T  # [64, BS]
    s_ = np.sin(freqs).T
    cosT = np.ascontiguousarray(np.concatenate([c, c], 0), dtype=np.float32)      # [128,BS]
    sinS = np.ascontiguousarray(np.concatenate([-s_, s_], 0), dtype=np.float32)   # signed

    hT = np.ascontiguousarray(hidden_states.reshape(BS, H).T)

    perm = np.zeros((128, 128), dtype=np.float32)
    for dp in range(128):
        perm[(dp + 64) % 128, dp] = 1.0  # out[dp] = in[(dp+64)%128]

    shared = {
        "hT": hT, "cosT": cosT, "sinS": sinS, "permP": perm,
        "ones128": np.ones((128, 128), dtype=np.float32),
    }
    if mask_mode == "causal":
        cm0, cm1 = _causal_patterns()
        shared["cmask0"] = cm0
        shared["cmask1"] = cm1
    if mask_mode == "generic":
        shared["maskT"] = np.ascontiguousarray(
            np.transpose(m, (0, 2, 1)) / SCALE
        ).astype(np.float32)

    in_maps = []
    for c_id in range(8):
        r = c_id * DLOC
        wqk = np.vstack([qkv_w[r:r + DLOC], qkv_w[H + r:H + r + DLOC]])      # [512, H]
        wv = qkv_w[2 * H + r: 2 * H + r + DLOC]                               # [256, H]
        bqk = np.concatenate([qkv_b[r:r + DLOC], qkv_b[H + r:H + r + DLOC]])  # [512]
        im = dict(shared)
        im["wqkT"] = np.ascontiguousarray(wqk.T)
        im["wvT"] = np.ascontiguousarray(wv.T)
        im["owT"] = np.ascontiguousarray(o_w[:, r:r + DLOC].T)
        im["bqkT"] = np.ascontiguousarray(bqk.reshape(4, 128).T)
        in_maps.append(im)
    post_bias = qkv_b[2 * H:3 * H] @ o_w.T  # [H], exact since sum(probs)=1
    return mask_mode, in_maps, post_bias


def kernel(**inputs) -> np.ndarray:
    from concourse.bass_utils import run_bass_kernel_spmd

    mask_mode, in_maps, post_bias = _host_prep(**inputs)
    nc = build_kernel(mask_mode)
    res = run_bass_kernel_spmd(nc, in_maps, core_ids=list(range(8)), trace=False)
    out = np.zeros((BS, H), dtype=np.float64)
    for r in res.results:
        out += r["outP"].astype(np.float64)
    out += post_bias.astype(np.float64)[None, :]
    return out.astype(np.float32).reshape(B, S, H)
